# revision 27
# baseline (speedup 1.0000x reference)
"""Causal single-head attention on 8 Trainium2 NeuronCores.

Problem: x[B=4,T=4096,C=512] fp32, Wk/Wq/Wv[C,H=64] -> out[B,T,H].

Sharding: 2 cores per batch element. Within a pair, the KEY tiles (128 keys
each, 32 tiles) are interleaved by parity: core parity p owns key tiles
{p, p+2, p+4, ...}. Each core computes, for ALL queries of its batch, the
unnormalized partial softmax numerator (sum_k exp(s) * v) and denominator
(sum_k exp(s)) over its own keys only; the host sums the two partials and
divides. exp() without max-subtraction is safe here (scores ~ N(0,1)).

Every core's program is byte-identical (SPMD): causal structure is identical
for both parities, and all parity/batch differences live in the DMA'd data
(xt columns permuted to [own-parity key tiles | rest], additive diag masks,
host un-permutes output columns).

The steady state is a balanced ACT/PE pipeline: back-to-back [128,1024]
exps stream at ~1000ns each (36 of them, one per key-tile-pair x
query-block); per pair the PE owes S (one concurrent row-packed pair,
~213ns) + AV (2 matmuls, ~430ns), leaving ~360ns of slack that the
projection work must fit inside. Everything is scheduled around keeping
the exp stream dense:
  - xt arrives in 256-col per-cc chunks on the sync (kv) / gpsimd (oth)
    rings, first wave spread across all three rings (per-HWDGE-queue BW is
    only ~25GB/s, ring issue rate ~650ns/DMA), ordered by first consumer
  - projections run as deadline-scheduled filler tasks woven between
    attention pairs (512-col MM groups mid-stream to halve LDWEIGHTS
    overhead; 256-col for the latency-critical preamble)
  - S pair matmuls use tile_position row-packing (concurrent, 64-contract)
  - the causal mask is multiplicative fp16, applied post-exp on the DVE so
    the ACT stream never waits on it; block 7 drains diag-first so the
    final pair has no mask on its critical path
  - output partials are fp16, block-major in DRAM (one DMA per block on
    gpsimd); the LAST block is drained as two half casts whose DMAs ride
    the scalar and sync rings so the two HWDGE descriptor generations run
    in parallel off gpsimd's queue walk

Compute fp16 (PE full rate), PSUM accumulation fp32.
"""

import os
from collections import deque
import numpy as np

B, T, C, H = 4, 4096, 512, 64
NKT = T // 128          # 32 natural key tiles per batch
NLOC = NKT // 2         # 16 local key tiles per core
NP = NLOC // 2          # 8 local tile-pairs == 8 query blocks
QB = T // 512           # 8 query blocks
SCALE = float(H) ** -0.5

_CACHE = {}
LAST_RESULTS = None


def _build_program():
    from contextlib import ExitStack
    import concourse.tile as tile
    from concourse import bacc, mybir
    from concourse.masks import make_identity

    F32 = mybir.dt.float32
    F16 = mybir.dt.float16
    EXP = mybir.ActivationFunctionType.Exp

    nc = bacc.Bacc("TRN2", target_bir_lowering=False, debug=False,
                   num_devices=8)

    xt = nc.dram_tensor("xt", (C, T), F16, kind="ExternalInput").ap()
    wkk = nc.dram_tensor("wkk", (128, 512), F16, kind="ExternalInput").ap()
    wqq = nc.dram_tensor("wqq", (128, 512), F16, kind="ExternalInput").ap()
    wv = nc.dram_tensor("wv", (128, 4 * H), F16, kind="ExternalInput").ap()
    masks = nc.dram_tensor("masks", (128, 1024), F16, kind="ExternalInput").ap()
    opart = nc.dram_tensor("opart", (QB * 65, 512), F16,
                           kind="ExternalOutput").ap()

    with tile.TileContext(nc) as tc, ExitStack() as ctx:
        persist = ctx.enter_context(tc.tile_pool(name="persist", bufs=1))
        vst_p = ctx.enter_context(tc.tile_pool(name="vst", bufs=2))
        p_pool = ctx.enter_context(tc.tile_pool(name="pp", bufs=3))
        out_p = ctx.enter_context(tc.tile_pool(name="outp", bufs=4))
        ps_s = ctx.enter_context(tc.tile_pool(name="pss", bufs=2, space="PSUM"))
        ps_o = ctx.enter_context(tc.tile_pool(name="pso", bufs=2, space="PSUM"))
        ps_j = ctx.enter_context(tc.tile_pool(name="psj", bufs=2, space="PSUM"))

        # ---- persistent SBUF ----
        wkk_sb = persist.tile([128, 512], F16)
        wqq_sb = persist.tile([128, 512], F16)
        wv_sb = persist.tile([128, 4 * H], F16)
        mask_sb = persist.tile([128, 1024], F16)
        kTq_sb = persist.tile([128, NLOC * 128], F16)
        qTq_sb = persist.tile([128, T], F16)
        v_sb = persist.tile([128, NLOC * 65], F16)
        ident = persist.tile([64, 64], F16)
        xt_sb = persist.tile([128, 4 * T], F16)

        zeros16 = persist.tile([128, 16], F16)
        warm_sc = persist.tile([64, 64], F16)

        # local init FIRST: all gpsimd-engine work must precede the gpsimd
        # DMA issues (in-order engine queue), or the PE warmup and identity
        # wait ~13us for the oth xt waves to issue
        nc.gpsimd.memset(zeros16[:], 0.0)
        nc.gpsimd.memset(warm_sc[:], 0.0)
        make_identity(nc, ident[:])
        # ones column of [V|1]: out = in*0 + 1 (also triggers ACT table load)
        v_ones = v_sb[:].rearrange("p (l e) -> p l e", e=65)[:, :, 64:65]
        nc.scalar.activation(v_ones, zeros16[:],
                             mybir.ActivationFunctionType.Copy,
                             bias=1.0, scale=0.0)

        # Per-HWDGE-queue bandwidth is only ~25 GB/s and each ring issues
        # one DMA per ~650ns, so the first wave (kv/oth cols 0:256 + weights)
        # is spread across all three rings for minimum time-to-first-S;
        # later chunks follow in consumption order.
        def kvc(w, cc, eng):
            eng.dma_start(
                xt_sb[:, cc * T + 256 * w:cc * T + 256 * (w + 1)],
                xt[128 * cc:128 * (cc + 1), 256 * w:256 * (w + 1)])

        def othc(w, cc, eng):
            c0 = 2048 + 256 * w
            eng.dma_start(xt_sb[:, cc * T + c0:cc * T + c0 + 256],
                          xt[128 * cc:128 * (cc + 1), c0:c0 + 256])

        nc.sync.dma_start(wkk_sb[:, 0:256], wkk[:, 0:256])
        nc.gpsimd.dma_start(wkk_sb[:, 256:512], wkk[:, 256:512])
        nc.scalar.dma_start(wqq_sb[:, 0:256], wqq[:, 0:256])
        nc.scalar.dma_start(wqq_sb[:, 256:512], wqq[:, 256:512])
        kvc(0, 0, nc.sync)
        kvc(0, 1, nc.sync)
        kvc(0, 2, nc.sync)
        nc.sync.dma_start(wv_sb[:], wv[:])
        kvc(0, 3, nc.gpsimd)
        othc(0, 0, nc.gpsimd)
        othc(0, 1, nc.gpsimd)
        othc(0, 2, nc.gpsimd)
        othc(0, 3, nc.gpsimd)
        # scalar-ring assist: ONLY the two last-arriving oth wave-1 chunks
        # (the gate of the first big exp gap), issued before the masks so
        # their data drains first; all descriptor generation completes well
        # before the first exp enters the ACT queue. Do NOT put more here —
        # 8 mid-stream chunks on this ring measured +8us (v5).
        othc(1, 2, nc.scalar)
        othc(1, 3, nc.scalar)
        nc.scalar.dma_start(mask_sb[:, 0:512], masks[:, 0:512])
        nc.scalar.dma_start(mask_sb[:, 512:1024], masks[:, 512:1024])
        othc(3, 2, nc.scalar)
        othc(3, 3, nc.scalar)
        # Ring assignment follows consumer DEADLINES, not data kind: the oth
        # waves feed qp_w whose deadlines are early (slots 0-8), so they get
        # the sync ring (starts issuing immediately); kv waves feed kp_w/vpm_w
        # with much later deadlines and can absorb gpsimd's slower queue.
        for w in range(1, 8):
            for cc in range(4):
                if w in (1, 3) and cc >= 2:
                    continue
                othc(w, cc, nc.sync)
            for cc in range(4):
                kvc(w, cc, nc.gpsimd)

        # PE warmup during the DMA preamble so the HAM clock-gate is released
        pwarm = ps_o.tile([64, 64], F32, tag="po")
        for _w in range(28):
            nc.tensor.matmul(pwarm[:], warm_sc[:], warm_sc[:],
                             start=True, stop=True, skip_group_check=True)

        # ---- projection micro-tasks (256 output columns each) ----
        def warm_fill(n):
            # keepalive matmuls between DMA-gated preamble MMs: the in-order
            # PE queue idles while the next chunk lands; these bridge the
            # HAM activity window so the ramp doesn't run at 1.2 GHz
            for _ in range(n):
                nc.tensor.matmul(pwarm[:], warm_sc[:], warm_sc[:],
                                 start=True, stop=True,
                                 skip_group_check=True)

        def kp(l, fill=0):
            # K^T for local tiles 2l, 2l+1 -> kTq cols 256l..256l+256,
            # duplicated across both partition halves ([Wk|Wk] stationary)
            pkk = ps_j.tile([128, 256], F32, tag="pj")
            for cc in range(4):
                nc.tensor.matmul(
                    pkk[:], wkk_sb[:, 128 * cc:128 * (cc + 1)],
                    xt_sb[:, cc * T + 256 * l:cc * T + 256 * (l + 1)],
                    start=(cc == 0), stop=(cc == 3))
                warm_fill(fill)
            nc.vector.tensor_copy(kTq_sb[:, 256 * l:256 * (l + 1)], pkk[:])

        vt_c = {}

        def vpm(l):
            pvv = ps_j.tile([64, 256], F32, tag="pj")
            for cc in range(4):
                nc.tensor.matmul(
                    pvv[:], wv_sb[:, H * cc:H * (cc + 1)],
                    xt_sb[:, cc * T + 256 * l:cc * T + 256 * (l + 1)],
                    start=(cc == 0), stop=(cc == 3))
            vt_st = vst_p.tile([64, 256], F16, tag="vst")
            nc.vector.tensor_copy(vt_st[:], pvv[:])
            vt_c[l] = vt_st[:]

        def vpt(l):
            vt_st = vt_c.pop(l)
            pv = ps_j.tile([128, 128], F16, tag="pj")
            for j in range(2):
                nc.tensor.transpose(pv[:, 64 * j:64 * (j + 1)],
                                    vt_st[:, 128 * j:128 * (j + 1)], ident[:])
            vv = v_sb[:].rearrange("p (l e) -> p l e", e=65)
            nc.vector.tensor_copy(
                vv[:, 2 * l:2 * l + 2, 0:64],
                pv[:].rearrange("p (j e) -> p j e", e=64))

        def qp(qb, half, fill=0):
            # Q^T for query block qb (kv half or oth half), duplicated
            # across partition halves ([Wq|Wq] stationary)
            off = 2048 * half + 256 * qb
            pqq = ps_j.tile([128, 256], F32, tag="pj")
            for cc in range(4):
                nc.tensor.matmul(
                    pqq[:], wqq_sb[:, 128 * cc:128 * (cc + 1)],
                    xt_sb[:, cc * T + off:cc * T + off + 256],
                    start=(cc == 0), stop=(cc == 3))
                warm_fill(fill)
            nc.vector.tensor_copy(qTq_sb[:, off:off + 256], pqq[:])

        # wide (512-col) task variants: half the LDWEIGHTS per output column
        xt_v = xt_sb[:].rearrange("p (cc half c) -> p cc half c", cc=4, half=2)
        qT_h = qTq_sb[:].rearrange("p (half c) -> p half c", half=2)

        def qp_w(qb):
            # both halves of query block qb in one MM group: moving is the
            # 2-region AP [kv cols | oth cols], out [128, 512]
            pqq = ps_j.tile([128, 512], F32, tag="pj")
            for cc in range(4):
                nc.tensor.matmul(
                    pqq[:], wqq_sb[:, 128 * cc:128 * (cc + 1)],
                    xt_v[:, cc, :, 256 * qb:256 * (qb + 1)],
                    start=(cc == 0), stop=(cc == 3))
            nc.vector.tensor_copy(qT_h[:, :, 256 * qb:256 * (qb + 1)], pqq[:])

        def kp_w(g):
            # K^T for local tiles 4g..4g+3 (kTq cols 512g..512g+512)
            pkk = ps_j.tile([128, 512], F32, tag="pj")
            for cc in range(4):
                nc.tensor.matmul(
                    pkk[:], wkk_sb[:, 128 * cc:128 * (cc + 1)],
                    xt_sb[:, cc * T + 512 * g:cc * T + 512 * (g + 1)],
                    start=(cc == 0), stop=(cc == 3))
            nc.vector.tensor_copy(kTq_sb[:, 512 * g:512 * (g + 1)], pkk[:])

        def vpm_w(g):
            pvv = ps_j.tile([64, 512], F32, tag="pj")
            for cc in range(4):
                nc.tensor.matmul(
                    pvv[:], wv_sb[:, H * cc:H * (cc + 1)],
                    xt_sb[:, cc * T + 512 * g:cc * T + 512 * (g + 1)],
                    start=(cc == 0), stop=(cc == 3))
            vt_st = vst_p.tile([64, 512], F16, tag="vst")
            nc.vector.tensor_copy(vt_st[:], pvv[:])
            vt_c[2 * g] = vt_st[:, 0:256]
            vt_c[2 * g + 1] = vt_st[:, 256:512]

        def t_qp_w(qb):
            return lambda: qp_w(qb)

        def t_kp_w(g):
            return lambda: kp_w(g)

        def t_vpm_w(g):
            return lambda: vpm_w(g)

        # half-granularity wide tasks: 2 MMs per slot (~fits the per-pair PE
        # slack) with the accumulation group and PSUM tile spanning 2 slots
        half_t = {}

        def qpw_a(qb):
            pqq = ps_j.tile([128, 512], F32, tag="pj")
            half_t[("q", qb)] = pqq
            for cc in range(2):
                nc.tensor.matmul(
                    pqq[:], wqq_sb[:, 128 * cc:128 * (cc + 1)],
                    xt_v[:, cc, :, 256 * qb:256 * (qb + 1)],
                    start=(cc == 0), stop=False)

        def qpw_b(qb):
            pqq = half_t.pop(("q", qb))
            for cc in range(2, 4):
                nc.tensor.matmul(
                    pqq[:], wqq_sb[:, 128 * cc:128 * (cc + 1)],
                    xt_v[:, cc, :, 256 * qb:256 * (qb + 1)],
                    start=False, stop=(cc == 3))
            nc.vector.tensor_copy(qT_h[:, :, 256 * qb:256 * (qb + 1)], pqq[:])

        def kpw_a(g):
            pkk = ps_j.tile([128, 512], F32, tag="pj")
            half_t[("k", g)] = pkk
            for cc in range(2):
                nc.tensor.matmul(
                    pkk[:], wkk_sb[:, 128 * cc:128 * (cc + 1)],
                    xt_sb[:, cc * T + 512 * g:cc * T + 512 * (g + 1)],
                    start=(cc == 0), stop=False)

        def kpw_b(g):
            pkk = half_t.pop(("k", g))
            for cc in range(2, 4):
                nc.tensor.matmul(
                    pkk[:], wkk_sb[:, 128 * cc:128 * (cc + 1)],
                    xt_sb[:, cc * T + 512 * g:cc * T + 512 * (g + 1)],
                    start=False, stop=(cc == 3))
            nc.vector.tensor_copy(kTq_sb[:, 512 * g:512 * (g + 1)], pkk[:])

        def vpmw_a(g):
            pvv = ps_j.tile([64, 512], F32, tag="pj")
            half_t[("v", g)] = pvv
            for cc in range(2):
                nc.tensor.matmul(
                    pvv[:], wv_sb[:, H * cc:H * (cc + 1)],
                    xt_sb[:, cc * T + 512 * g:cc * T + 512 * (g + 1)],
                    start=(cc == 0), stop=False)

        def vpmw_b(g):
            pvv = half_t.pop(("v", g))
            for cc in range(2, 4):
                nc.tensor.matmul(
                    pvv[:], wv_sb[:, H * cc:H * (cc + 1)],
                    xt_sb[:, cc * T + 512 * g:cc * T + 512 * (g + 1)],
                    start=False, stop=(cc == 3))
            vt_st = vst_p.tile([64, 512], F16, tag="vst")
            nc.vector.tensor_copy(vt_st[:], pvv[:])
            vt_c[2 * g] = vt_st[:, 0:256]
            vt_c[2 * g + 1] = vt_st[:, 256:512]

        # cc-outer PAIRED wide tasks: two groups' partials per cc so that
        # consecutive matmuls share the stationary W chunk — walrus emits one
        # LDWEIGHTS for a run of same-stationary matmuls (the warmup loop
        # shows this dedup in the trace), halving proj LDWEIGHTS overhead
        def qpw2a(q1, q2):
            for qb in (q1, q2):
                pqq2 = ps_j.tile([128, 512], F32, tag="pj")
                half_t[("q", qb)] = pqq2
            for cc in range(2):
                for qb in (q1, q2):
                    nc.tensor.matmul(
                        half_t[("q", qb)][:],
                        wqq_sb[:, 128 * cc:128 * (cc + 1)],
                        xt_v[:, cc, :, 256 * qb:256 * (qb + 1)],
                        start=(cc == 0), stop=False)

        def qpw2b(q1, q2):
            for cc in range(2, 4):
                for qb in (q1, q2):
                    nc.tensor.matmul(
                        half_t[("q", qb)][:],
                        wqq_sb[:, 128 * cc:128 * (cc + 1)],
                        xt_v[:, cc, :, 256 * qb:256 * (qb + 1)],
                        start=False, stop=(cc == 3))
            for qb in (q1, q2):
                pqq = half_t.pop(("q", qb))
                nc.vector.tensor_copy(qT_h[:, :, 256 * qb:256 * (qb + 1)],
                                      pqq[:])

        def kpw2a(g1, g2):
            for g in (g1, g2):
                pkk2 = ps_j.tile([128, 512], F32, tag="pj")
                half_t[("k", g)] = pkk2
            for cc in range(2):
                for g in (g1, g2):
                    nc.tensor.matmul(
                        half_t[("k", g)][:],
                        wkk_sb[:, 128 * cc:128 * (cc + 1)],
                        xt_sb[:, cc * T + 512 * g:cc * T + 512 * (g + 1)],
                        start=(cc == 0), stop=False)

        def kpw2b(g1, g2):
            for cc in range(2, 4):
                for g in (g1, g2):
                    nc.tensor.matmul(
                        half_t[("k", g)][:],
                        wkk_sb[:, 128 * cc:128 * (cc + 1)],
                        xt_sb[:, cc * T + 512 * g:cc * T + 512 * (g + 1)],
                        start=False, stop=(cc == 3))
            for g in (g1, g2):
                pkk = half_t.pop(("k", g))
                nc.vector.tensor_copy(kTq_sb[:, 512 * g:512 * (g + 1)],
                                      pkk[:])

        def vpmw2a(g1, g2):
            for g in (g1, g2):
                pvv2 = ps_j.tile([64, 512], F32, tag="pj")
                half_t[("v", g)] = pvv2
            for cc in range(2):
                for g in (g1, g2):
                    nc.tensor.matmul(
                        half_t[("v", g)][:],
                        wv_sb[:, H * cc:H * (cc + 1)],
                        xt_sb[:, cc * T + 512 * g:cc * T + 512 * (g + 1)],
                        start=(cc == 0), stop=False)

        def vpmw2b(g1, g2):
            for cc in range(2, 4):
                for g in (g1, g2):
                    nc.tensor.matmul(
                        half_t[("v", g)][:],
                        wv_sb[:, H * cc:H * (cc + 1)],
                        xt_sb[:, cc * T + 512 * g:cc * T + 512 * (g + 1)],
                        start=False, stop=(cc == 3))
            for g in (g1, g2):
                pvv = half_t.pop(("v", g))
                vt_st = vst_p.tile([64, 512], F16, tag="vst")
                nc.vector.tensor_copy(vt_st[:], pvv[:])
                vt_c[2 * g] = vt_st[:, 0:256]
                vt_c[2 * g + 1] = vt_st[:, 256:512]

        def t_h2(fn, a, b):
            return lambda: fn(a, b)

        def t_h(fn, x):
            return lambda: fn(x)

        def t_kp(l):
            return lambda: kp(l)

        def t_vpm(l):
            return lambda: vpm(l)

        def t_vpt(l):
            return lambda: vpt(l)

        def t_qp(b, half):
            return lambda: qp(b, half)

        slot_map = {
            0: [t_qp_w(1), t_vpt(0)],
            1: [t_kp(1), t_qp_w(2)],
            2: [t_vpm(1)],
            3: [t_qp_w(3), t_vpt(1)],
            4: [t_kp_w(1)],
            5: [t_vpm_w(1), t_vpt(2)],
            6: [t_vpt(3)],
            7: [t_h(qpw_a, 4)],
            8: [t_h(qpw_b, 4)],
            9: [t_h(kpw_a, 2)],
            10: [t_h(kpw_b, 2)],
            11: [t_h(vpmw_a, 2)],
            12: [t_h(vpmw_b, 2)],
            13: [t_h(qpw_a, 5)],
            14: [t_h(qpw_b, 5), t_vpt(4)],
            15: [t_vpt(5)],
            16: [t_h(kpw_a, 3)],
            17: [t_h(kpw_b, 3)],
            18: [t_h(qpw_a, 6)],
            19: [t_h(qpw_b, 6)],
            20: [t_h(vpmw_a, 3)],
            21: [t_h(vpmw_b, 3)],
            22: [t_vpt(6)],
            23: [t_vpt(7)],
            25: [t_h(qpw_a, 7)],
            26: [t_h(qpw_b, 7)],
        }

        # ---- attention: flat S -> exp -> AV pipeline over all 36 pairs ----
        qT_v = qTq_sb[:].rearrange("p (half c) -> p half c", half=2)
        po_t = {}

        def emit_S(qb, lp):
            q_lo = qT_v[0:64, :, 256 * qb:256 * (qb + 1)]
            q_hi = qT_v[64:128, :, 256 * qb:256 * (qb + 1)]
            ps = ps_s.tile([128, 1024], F32, tag="s")
            c0 = 256 * lp
            nc.tensor.matmul(ps[:, 0:512], kTq_sb[0:64, c0:c0 + 128],
                             q_lo, start=True, stop=True,
                             tile_position=(0, 0))
            nc.tensor.matmul(ps[:, 512:1024], kTq_sb[64:128, c0 + 128:c0 + 256],
                             q_hi, start=True, stop=True,
                             tile_position=(64, 0))
            return ps

        def emit_exp(qb, lp, ps):
            p_sb = p_pool.tile([128, 1024], F16, tag="p")
            nc.scalar.activation(p_sb[:], ps[:], EXP, scale=SCALE)
            if lp == qb:  # diagonal pair: multiplicative causal mask, DVE,
                # after exp so the ACT stream never waits on it
                nc.vector.tensor_mul(p_sb[:], p_sb[:], mask_sb[:])
            return p_sb

        def emit_AV(qb, lp, p_sb, first, last):
            if first:
                po_new = ps_o.tile([65, 512], F32, tag="po")
                po_t[qb] = po_new
            po = po_t[qb]
            for h in range(2):
                l = 2 * lp + h
                nc.tensor.matmul(po[:], v_sb[:, 65 * l:65 * (l + 1)],
                                 p_sb[:, 512 * h:512 * (h + 1)],
                                 start=(first and h == 0), stop=(last and h == 1))
            if last:  # block done
                po = po_t.pop(qb)
                if qb == QB - 1:
                    # final block: three third casts + three idle rings so
                    # all descriptor generations run in parallel (gpsimd's
                    # queue is clean now that early outputs ride sync)
                    o1 = out_p.tile([65, 172], F16, tag="o1")
                    nc.vector.tensor_copy(o1[:], po[:, 0:172])
                    nc.scalar.dma_start(opart[65 * qb:65 * (qb + 1), 0:172],
                                        o1[:])
                    o2 = out_p.tile([65, 172], F16, tag="o2")
                    nc.vector.tensor_copy(o2[:], po[:, 172:344])
                    nc.sync.dma_start(opart[65 * qb:65 * (qb + 1), 172:344],
                                      o2[:])
                    o3 = out_p.tile([65, 168], F16, tag="o3")
                    nc.vector.tensor_copy(o3[:], po[:, 344:512])
                    nc.gpsimd.dma_start(opart[65 * qb:65 * (qb + 1), 344:512],
                                        o3[:])
                else:
                    o_sb = out_p.tile([65, 512], F16, tag="o")
                    nc.vector.tensor_copy(o_sb[:], po[:])
                    # sync ring is idle after the input waves (~27us) and has
                    # no compute sequencer: outputs there keep gpsimd's queues
                    # clean for the teardown's queue walk
                    nc.sync.dma_start(opart[65 * qb:65 * (qb + 1), :],
                                      o_sb[:])

        s_q = deque()   # (qb, lp, first, last, ps) awaiting exp
        e_q = deque()   # (qb, lp, first, last, p_sb) awaiting AV

        def pump():
            if len(s_q) >= 2:
                qb, lp, fi, la, ps = s_q.popleft()
                e_q.append((qb, lp, fi, la, emit_exp(qb, lp, ps)))
            if len(e_q) >= 2:
                qb, lp, fi, la, p_sb = e_q.popleft()
                emit_AV(qb, lp, p_sb, fi, la)

        # preamble projections: only what gates S(0,0); V rides slot 0
        kp(0)
        qp(0, 0)
        qp(0, 1)
        vpm(0)

        pairs = []
        for qb in range(QB - 1):
            for lp in range(qb + 1):
                pairs.append((qb, lp, lp == 0, lp == qb))
        for j, lp in enumerate(range(QB - 1, -1, -1)):
            pairs.append((QB - 1, lp, j == 0, j == QB - 1))
        for p_idx, (qb, lp, fi, la) in enumerate(pairs):
            s_q.append((qb, lp, fi, la, emit_S(qb, lp)))
            pump()
            for fn in slot_map.pop(p_idx, []):
                fn()
        assert not slot_map
        while s_q or e_q:
            if s_q:
                qb, lp, fi, la, ps = s_q.popleft()
                e_q.append((qb, lp, fi, la, emit_exp(qb, lp, ps)))
            if e_q:
                qb, lp, fi, la, p_sb = e_q.popleft()
                emit_AV(qb, lp, p_sb, fi, la)

    nc.compile()
    return nc


def _prep_inputs(x, Wk, Wq, Wv):
    """Per-core input marshalling (layout + fp16 cast, no math)."""
    def swz(w):
        # [C, m] -> [128, 4*m]: chunk cc (rows 128cc..) at free cols m*cc..
        m = w.shape[1]
        return np.ascontiguousarray(
            w.reshape(4, 128, m).transpose(1, 0, 2).reshape(128, 4 * m)
        ).astype(np.float16)

    wkk = swz(np.concatenate([Wk, Wk], axis=1))
    wqq = swz(np.concatenate([Wq, Wq], axis=1))
    wv = swz(Wv)
    mask_cache = {}
    in_maps = []
    for core in range(8):
        b, par = core // 2, core % 2
        xT = np.ascontiguousarray(x[b].T).astype(np.float16)   # [C, T]
        tiles = xT.reshape(C, NKT, 128)
        kv = tiles[:, par::2, :].reshape(C, NLOC * 128)
        oth = tiles[:, 1 - par::2, :].reshape(C, NLOC * 128)
        xt_perm = np.ascontiguousarray(np.concatenate([kv, oth], axis=1))

        if par not in mask_cache:
            J = [par, 2 + par, 1 - par, 3 - par]
            m = np.zeros((128, 1024), np.float16)
            ks = np.arange(128)[:, None]
            qr = np.arange(128)[None, :]
            for mi, off in enumerate((par, 2 + par)):
                for s in range(4):
                    cond = (128 * off + ks) <= (128 * J[s] + qr)
                    m[:, 512 * mi + 128 * s:512 * mi + 128 * (s + 1)] = \
                        np.where(cond, 1.0, 0.0).astype(np.float16)
            mask_cache[par] = m
        in_maps.append({"xt": xt_perm, "wkk": wkk, "wqq": wqq, "wv": wv,
                        "masks": mask_cache[par]})
    return in_maps


def _combine(results):
    """Un-permute query columns, sum partials across the core pairs, divide."""
    out = np.empty((B, T, H), np.float32)
    for b in range(4):
        nats = []
        for par in range(2):
            J = [par, 2 + par, 1 - par, 3 - par]
            r = results[2 * b + par]["opart"].astype(np.float32)
            r = r.reshape(QB, 65, 4, 128)
            nat = np.empty_like(r)
            for s in range(4):
                nat[:, :, J[s], :] = r[:, :, s, :]
            nats.append(nat.transpose(1, 0, 2, 3).reshape(65, T))
        num = nats[0][:64] + nats[1][:64]
        den = nats[0][64] + nats[1][64]
        out[b] = (num / den[None, :]).T
    return out


def kernel(x, Wk, Wq, Wv):
    global LAST_RESULTS
    from concourse.bass_utils import run_bass_kernel_spmd

    if "nc" not in _CACHE:
        _CACHE["nc"] = _build_program()
    nc = _CACHE["nc"]

    in_maps = _prep_inputs(np.asarray(x, np.float32), np.asarray(Wk),
                           np.asarray(Wq), np.asarray(Wv))
    trace = bool(int(os.environ.get("ATTN_TRACE", "0")))
    res = run_bass_kernel_spmd(nc, in_maps, core_ids=list(range(8)),
                               trace=trace)
    LAST_RESULTS = res
    return _combine(res.results)


if __name__ == "__main__":
    rng = np.random.default_rng(0)
    x = rng.standard_normal((B, T, C), dtype=np.float32)
    Wk = (rng.standard_normal((C, H)) * C ** -0.5).astype(np.float32)
    Wq = (rng.standard_normal((C, H)) * C ** -0.5).astype(np.float32)
    Wv = (rng.standard_normal((C, H)) * C ** -0.5).astype(np.float32)
    out = kernel(x, Wk, Wq, Wv)
    k = x @ Wk; q = x @ Wq; v = x @ Wv
    s = np.einsum('bqh,bkh->bqk', q, k) * SCALE
    mask = np.tril(np.ones((T, T), dtype=bool))
    s = np.where(mask, s, -np.inf)
    p = np.exp(s - s.max(-1, keepdims=True))
    p /= p.sum(-1, keepdims=True)
    ref = np.einsum('bqk,bkh->bqh', p, v)
    err = np.abs(out - ref).max() / np.abs(ref).max()
    print("rel err vs numpy:", err)



# revision 28
# speedup vs baseline: 1.0336x; 1.0336x over previous
"""Causal single-head attention on 8 Trainium2 NeuronCores.

Problem: x[B=4,T=4096,C=512] fp32, Wk/Wq/Wv[C,H=64] -> out[B,T,H].

Sharding: 2 cores per batch element. Within a pair, the KEY tiles (128 keys
each, 32 tiles) are interleaved by parity: core parity p owns key tiles
{p, p+2, p+4, ...}. Each core computes, for ALL queries of its batch, the
unnormalized partial softmax numerator (sum_k exp(s) * v) and denominator
(sum_k exp(s)) over its own keys only; the host sums the two partials and
divides. exp() without max-subtraction is safe here (scores ~ N(0,1)).

Every core's program is byte-identical (SPMD): causal structure is identical
for both parities, and all parity/batch differences live in the DMA'd data
(xt columns permuted to [own-parity key tiles | rest], additive diag masks,
host un-permutes output columns).

The steady state is a balanced ACT/PE pipeline: back-to-back [128,1024]
exps stream at ~1000ns each (36 of them, one per key-tile-pair x
query-block); per pair the PE owes S (one concurrent row-packed pair,
~213ns) + AV (2 matmuls, ~430ns), leaving ~360ns of slack that the
projection work must fit inside. Everything is scheduled around keeping
the exp stream dense:
  - xt arrives in 256-col per-cc chunks on the sync (kv) / gpsimd (oth)
    rings, first wave spread across all three rings (per-HWDGE-queue BW is
    only ~25GB/s, ring issue rate ~650ns/DMA), ordered by first consumer
  - projections run as deadline-scheduled filler tasks woven between
    attention pairs (512-col MM groups mid-stream to halve LDWEIGHTS
    overhead; 256-col for the latency-critical preamble)
  - S pair matmuls use tile_position row-packing (concurrent, 64-contract)
  - the causal mask is multiplicative fp16, applied post-exp on the DVE so
    the ACT stream never waits on it; block 7 drains diag-first so the
    final pair has no mask on its critical path
  - output partials are fp16, block-major in DRAM (one DMA per block on
    gpsimd); the LAST block is drained as two half casts whose DMAs ride
    the scalar and sync rings so the two HWDGE descriptor generations run
    in parallel off gpsimd's queue walk

Compute fp16 (PE full rate), PSUM accumulation fp32.
"""

import os
from collections import deque
import numpy as np

B, T, C, H = 4, 4096, 512, 64
NKT = T // 128          # 32 natural key tiles per batch
NLOC = NKT // 2         # 16 local key tiles per core
NP = NLOC // 2          # 8 local tile-pairs == 8 query blocks
QB = T // 512           # 8 query blocks
SCALE = float(H) ** -0.5

_CACHE = {}
LAST_RESULTS = None


def _build_program():
    from contextlib import ExitStack
    import concourse.tile as tile
    from concourse import bacc, mybir
    from concourse.masks import make_identity

    F32 = mybir.dt.float32
    F16 = mybir.dt.float16
    EXP = mybir.ActivationFunctionType.Exp

    nc = bacc.Bacc("TRN2", target_bir_lowering=False, debug=False,
                   num_devices=8)

    xt = nc.dram_tensor("xt", (C, T), F16, kind="ExternalInput").ap()
    wkk = nc.dram_tensor("wkk", (128, 512), F16, kind="ExternalInput").ap()
    wqq = nc.dram_tensor("wqq", (128, 512), F16, kind="ExternalInput").ap()
    wv = nc.dram_tensor("wv", (128, 4 * H), F16, kind="ExternalInput").ap()
    masks = nc.dram_tensor("masks", (128, 1024), F16, kind="ExternalInput").ap()
    opart = nc.dram_tensor("opart", (QB * 65, 512), F16,
                           kind="ExternalOutput").ap()

    with tile.TileContext(nc) as tc, ExitStack() as ctx:
        persist = ctx.enter_context(tc.tile_pool(name="persist", bufs=1))
        vst_p = ctx.enter_context(tc.tile_pool(name="vst", bufs=2))
        p_pool = ctx.enter_context(tc.tile_pool(name="pp", bufs=3))
        out_p = ctx.enter_context(tc.tile_pool(name="outp", bufs=4))
        ps_s = ctx.enter_context(tc.tile_pool(name="pss", bufs=2, space="PSUM"))
        ps_o = ctx.enter_context(tc.tile_pool(name="pso", bufs=2, space="PSUM"))
        ps_j = ctx.enter_context(tc.tile_pool(name="psj", bufs=2, space="PSUM"))

        # ---- persistent SBUF ----
        wkk_sb = persist.tile([128, 512], F16)
        wqq_sb = persist.tile([128, 512], F16)
        wv_sb = persist.tile([128, 4 * H], F16)
        mask_sb = persist.tile([128, 1024], F16)
        kTq_sb = persist.tile([128, NLOC * 128], F16)
        qTq_sb = persist.tile([128, T], F16)
        v_sb = persist.tile([128, NLOC * 65], F16)
        ident = persist.tile([64, 64], F16)
        xt_sb = persist.tile([128, 4 * T], F16)

        zeros16 = persist.tile([128, 16], F16)
        warm_sc = persist.tile([64, 64], F16)

        # local init FIRST: all gpsimd-engine work must precede the gpsimd
        # DMA issues (in-order engine queue), or the PE warmup and identity
        # wait ~13us for the oth xt waves to issue
        nc.gpsimd.memset(zeros16[:], 0.0)
        nc.gpsimd.memset(warm_sc[:], 0.0)
        make_identity(nc, ident[:])
        # ones column of [V|1]: out = in*0 + 1 (also triggers ACT table load)
        v_ones = v_sb[:].rearrange("p (l e) -> p l e", e=65)[:, :, 64:65]
        nc.scalar.activation(v_ones, zeros16[:],
                             mybir.ActivationFunctionType.Copy,
                             bias=1.0, scale=0.0)

        # Per-HWDGE-queue bandwidth is only ~25 GB/s and each ring issues
        # one DMA per ~650ns, so the first wave (kv/oth cols 0:256 + weights)
        # is spread across all three rings for minimum time-to-first-S;
        # later chunks follow in consumption order.
        def kvc(w, cc, eng):
            eng.dma_start(
                xt_sb[:, cc * T + 256 * w:cc * T + 256 * (w + 1)],
                xt[128 * cc:128 * (cc + 1), 256 * w:256 * (w + 1)])

        def othc(w, cc, eng):
            c0 = 2048 + 256 * w
            eng.dma_start(xt_sb[:, cc * T + c0:cc * T + c0 + 256],
                          xt[128 * cc:128 * (cc + 1), c0:c0 + 256])

        nc.sync.dma_start(wkk_sb[:, 0:256], wkk[:, 0:256])
        nc.gpsimd.dma_start(wkk_sb[:, 256:512], wkk[:, 256:512])
        nc.scalar.dma_start(wqq_sb[:, 0:256], wqq[:, 0:256])
        nc.scalar.dma_start(wqq_sb[:, 256:512], wqq[:, 256:512])
        kvc(0, 0, nc.sync)
        kvc(0, 1, nc.sync)
        kvc(0, 2, nc.sync)
        nc.sync.dma_start(wv_sb[:], wv[:])
        kvc(0, 3, nc.gpsimd)
        othc(0, 0, nc.gpsimd)
        othc(0, 1, nc.gpsimd)
        othc(0, 2, nc.gpsimd)
        othc(0, 3, nc.gpsimd)
        # scalar-ring assist: ONLY the two last-arriving oth wave-1 chunks
        # (the gate of the first big exp gap), issued before the masks so
        # their data drains first; all descriptor generation completes well
        # before the first exp enters the ACT queue. Do NOT put more here —
        # 8 mid-stream chunks on this ring measured +8us (v5).
        othc(1, 2, nc.scalar)
        othc(1, 3, nc.scalar)
        nc.scalar.dma_start(mask_sb[:, 0:512], masks[:, 0:512])
        nc.scalar.dma_start(mask_sb[:, 512:1024], masks[:, 512:1024])
        othc(3, 2, nc.scalar)
        othc(3, 3, nc.scalar)
        # Ring assignment follows consumer DEADLINES, not data kind: the oth
        # waves feed qp_w whose deadlines are early (slots 0-8), so they get
        # the sync ring (starts issuing immediately); kv waves feed kp_w/vpm_w
        # with much later deadlines and can absorb gpsimd's slower queue.
        for w in range(1, 8):
            for cc in range(4):
                if w in (1, 3) and cc >= 2:
                    continue
                othc(w, cc, nc.sync)
            for cc in range(4):
                kvc(w, cc, nc.gpsimd)

        # PE warmup during the DMA preamble so the HAM clock-gate is released
        pwarm = ps_o.tile([64, 64], F32, tag="po")
        for _w in range(28):
            nc.tensor.matmul(pwarm[:], warm_sc[:], warm_sc[:],
                             start=True, stop=True, skip_group_check=True)

        # ---- projection micro-tasks (256 output columns each) ----
        def warm_fill(n):
            # keepalive matmuls between DMA-gated preamble MMs: the in-order
            # PE queue idles while the next chunk lands; these bridge the
            # HAM activity window so the ramp doesn't run at 1.2 GHz
            for _ in range(n):
                nc.tensor.matmul(pwarm[:], warm_sc[:], warm_sc[:],
                                 start=True, stop=True,
                                 skip_group_check=True)

        def kp(l, fill=0):
            # K^T for local tiles 2l, 2l+1 -> kTq cols 256l..256l+256,
            # duplicated across both partition halves ([Wk|Wk] stationary)
            pkk = ps_j.tile([128, 256], F32, tag="pj")
            for cc in range(4):
                nc.tensor.matmul(
                    pkk[:], wkk_sb[:, 128 * cc:128 * (cc + 1)],
                    xt_sb[:, cc * T + 256 * l:cc * T + 256 * (l + 1)],
                    start=(cc == 0), stop=(cc == 3))
                warm_fill(fill)
            nc.vector.tensor_copy(kTq_sb[:, 256 * l:256 * (l + 1)], pkk[:])

        vt_c = {}

        def vpm(l):
            pvv = ps_j.tile([64, 256], F32, tag="pj")
            for cc in range(4):
                nc.tensor.matmul(
                    pvv[:], wv_sb[:, H * cc:H * (cc + 1)],
                    xt_sb[:, cc * T + 256 * l:cc * T + 256 * (l + 1)],
                    start=(cc == 0), stop=(cc == 3))
            vt_st = vst_p.tile([64, 256], F16, tag="vst")
            nc.vector.tensor_copy(vt_st[:], pvv[:])
            vt_c[l] = vt_st[:]

        def vpt(l):
            vt_st = vt_c.pop(l)
            pv = ps_j.tile([128, 128], F16, tag="pj")
            for j in range(2):
                nc.tensor.transpose(pv[:, 64 * j:64 * (j + 1)],
                                    vt_st[:, 128 * j:128 * (j + 1)], ident[:])
            vv = v_sb[:].rearrange("p (l e) -> p l e", e=65)
            nc.vector.tensor_copy(
                vv[:, 2 * l:2 * l + 2, 0:64],
                pv[:].rearrange("p (j e) -> p j e", e=64))

        def qp(qb, half, fill=0):
            # Q^T for query block qb (kv half or oth half), duplicated
            # across partition halves ([Wq|Wq] stationary)
            off = 2048 * half + 256 * qb
            pqq = ps_j.tile([128, 256], F32, tag="pj")
            for cc in range(4):
                nc.tensor.matmul(
                    pqq[:], wqq_sb[:, 128 * cc:128 * (cc + 1)],
                    xt_sb[:, cc * T + off:cc * T + off + 256],
                    start=(cc == 0), stop=(cc == 3))
                warm_fill(fill)
            nc.vector.tensor_copy(qTq_sb[:, off:off + 256], pqq[:])

        # wide (512-col) task variants: half the LDWEIGHTS per output column
        xt_v = xt_sb[:].rearrange("p (cc half c) -> p cc half c", cc=4, half=2)
        qT_h = qTq_sb[:].rearrange("p (half c) -> p half c", half=2)

        def qp_w(qb):
            # both halves of query block qb in one MM group: moving is the
            # 2-region AP [kv cols | oth cols], out [128, 512]
            pqq = ps_j.tile([128, 512], F32, tag="pj")
            for cc in range(4):
                nc.tensor.matmul(
                    pqq[:], wqq_sb[:, 128 * cc:128 * (cc + 1)],
                    xt_v[:, cc, :, 256 * qb:256 * (qb + 1)],
                    start=(cc == 0), stop=(cc == 3))
            nc.vector.tensor_copy(qT_h[:, :, 256 * qb:256 * (qb + 1)], pqq[:])

        def kp_w(g):
            # K^T for local tiles 4g..4g+3 (kTq cols 512g..512g+512)
            pkk = ps_j.tile([128, 512], F32, tag="pj")
            for cc in range(4):
                nc.tensor.matmul(
                    pkk[:], wkk_sb[:, 128 * cc:128 * (cc + 1)],
                    xt_sb[:, cc * T + 512 * g:cc * T + 512 * (g + 1)],
                    start=(cc == 0), stop=(cc == 3))
            nc.vector.tensor_copy(kTq_sb[:, 512 * g:512 * (g + 1)], pkk[:])

        def vpm_w(g):
            pvv = ps_j.tile([64, 512], F32, tag="pj")
            for cc in range(4):
                nc.tensor.matmul(
                    pvv[:], wv_sb[:, H * cc:H * (cc + 1)],
                    xt_sb[:, cc * T + 512 * g:cc * T + 512 * (g + 1)],
                    start=(cc == 0), stop=(cc == 3))
            vt_st = vst_p.tile([64, 512], F16, tag="vst")
            nc.vector.tensor_copy(vt_st[:], pvv[:])
            vt_c[2 * g] = vt_st[:, 0:256]
            vt_c[2 * g + 1] = vt_st[:, 256:512]

        def t_qp_w(qb):
            return lambda: qp_w(qb)

        def t_kp_w(g):
            return lambda: kp_w(g)

        def t_vpm_w(g):
            return lambda: vpm_w(g)

        # half-granularity wide tasks: 2 MMs per slot (~fits the per-pair PE
        # slack) with the accumulation group and PSUM tile spanning 2 slots
        half_t = {}

        def qpw_a(qb):
            pqq = ps_j.tile([128, 512], F32, tag="pj")
            half_t[("q", qb)] = pqq
            for cc in range(2):
                nc.tensor.matmul(
                    pqq[:], wqq_sb[:, 128 * cc:128 * (cc + 1)],
                    xt_v[:, cc, :, 256 * qb:256 * (qb + 1)],
                    start=(cc == 0), stop=False)

        def qpw_b(qb):
            pqq = half_t.pop(("q", qb))
            for cc in range(2, 4):
                nc.tensor.matmul(
                    pqq[:], wqq_sb[:, 128 * cc:128 * (cc + 1)],
                    xt_v[:, cc, :, 256 * qb:256 * (qb + 1)],
                    start=False, stop=(cc == 3))
            nc.vector.tensor_copy(qT_h[:, :, 256 * qb:256 * (qb + 1)], pqq[:])

        def kpw_a(g):
            pkk = ps_j.tile([128, 512], F32, tag="pj")
            half_t[("k", g)] = pkk
            for cc in range(2):
                nc.tensor.matmul(
                    pkk[:], wkk_sb[:, 128 * cc:128 * (cc + 1)],
                    xt_sb[:, cc * T + 512 * g:cc * T + 512 * (g + 1)],
                    start=(cc == 0), stop=False)

        def kpw_b(g):
            pkk = half_t.pop(("k", g))
            for cc in range(2, 4):
                nc.tensor.matmul(
                    pkk[:], wkk_sb[:, 128 * cc:128 * (cc + 1)],
                    xt_sb[:, cc * T + 512 * g:cc * T + 512 * (g + 1)],
                    start=False, stop=(cc == 3))
            nc.vector.tensor_copy(kTq_sb[:, 512 * g:512 * (g + 1)], pkk[:])

        def vpmw_a(g):
            pvv = ps_j.tile([64, 512], F32, tag="pj")
            half_t[("v", g)] = pvv
            for cc in range(2):
                nc.tensor.matmul(
                    pvv[:], wv_sb[:, H * cc:H * (cc + 1)],
                    xt_sb[:, cc * T + 512 * g:cc * T + 512 * (g + 1)],
                    start=(cc == 0), stop=False)

        def vpmw_b(g):
            pvv = half_t.pop(("v", g))
            for cc in range(2, 4):
                nc.tensor.matmul(
                    pvv[:], wv_sb[:, H * cc:H * (cc + 1)],
                    xt_sb[:, cc * T + 512 * g:cc * T + 512 * (g + 1)],
                    start=False, stop=(cc == 3))
            vt_st = vst_p.tile([64, 512], F16, tag="vst")
            nc.vector.tensor_copy(vt_st[:], pvv[:])
            vt_c[2 * g] = vt_st[:, 0:256]
            vt_c[2 * g + 1] = vt_st[:, 256:512]

        # cc-outer PAIRED wide tasks: two groups' partials per cc so that
        # consecutive matmuls share the stationary W chunk — walrus emits one
        # LDWEIGHTS for a run of same-stationary matmuls (the warmup loop
        # shows this dedup in the trace), halving proj LDWEIGHTS overhead
        def qpw2a(q1, q2):
            for qb in (q1, q2):
                pqq2 = ps_j.tile([128, 512], F32, tag="pj")
                half_t[("q", qb)] = pqq2
            for cc in range(2):
                for qb in (q1, q2):
                    nc.tensor.matmul(
                        half_t[("q", qb)][:],
                        wqq_sb[:, 128 * cc:128 * (cc + 1)],
                        xt_v[:, cc, :, 256 * qb:256 * (qb + 1)],
                        start=(cc == 0), stop=False)

        def qpw2b(q1, q2):
            for cc in range(2, 4):
                for qb in (q1, q2):
                    nc.tensor.matmul(
                        half_t[("q", qb)][:],
                        wqq_sb[:, 128 * cc:128 * (cc + 1)],
                        xt_v[:, cc, :, 256 * qb:256 * (qb + 1)],
                        start=False, stop=(cc == 3))
            for qb in (q1, q2):
                pqq = half_t.pop(("q", qb))
                nc.vector.tensor_copy(qT_h[:, :, 256 * qb:256 * (qb + 1)],
                                      pqq[:])

        def kpw2a(g1, g2):
            for g in (g1, g2):
                pkk2 = ps_j.tile([128, 512], F32, tag="pj")
                half_t[("k", g)] = pkk2
            for cc in range(2):
                for g in (g1, g2):
                    nc.tensor.matmul(
                        half_t[("k", g)][:],
                        wkk_sb[:, 128 * cc:128 * (cc + 1)],
                        xt_sb[:, cc * T + 512 * g:cc * T + 512 * (g + 1)],
                        start=(cc == 0), stop=False)

        def kpw2b(g1, g2):
            for cc in range(2, 4):
                for g in (g1, g2):
                    nc.tensor.matmul(
                        half_t[("k", g)][:],
                        wkk_sb[:, 128 * cc:128 * (cc + 1)],
                        xt_sb[:, cc * T + 512 * g:cc * T + 512 * (g + 1)],
                        start=False, stop=(cc == 3))
            for g in (g1, g2):
                pkk = half_t.pop(("k", g))
                nc.vector.tensor_copy(kTq_sb[:, 512 * g:512 * (g + 1)],
                                      pkk[:])

        def vpmw2a(g1, g2):
            for g in (g1, g2):
                pvv2 = ps_j.tile([64, 512], F32, tag="pj")
                half_t[("v", g)] = pvv2
            for cc in range(2):
                for g in (g1, g2):
                    nc.tensor.matmul(
                        half_t[("v", g)][:],
                        wv_sb[:, H * cc:H * (cc + 1)],
                        xt_sb[:, cc * T + 512 * g:cc * T + 512 * (g + 1)],
                        start=(cc == 0), stop=False)

        def vpmw2b(g1, g2):
            for cc in range(2, 4):
                for g in (g1, g2):
                    nc.tensor.matmul(
                        half_t[("v", g)][:],
                        wv_sb[:, H * cc:H * (cc + 1)],
                        xt_sb[:, cc * T + 512 * g:cc * T + 512 * (g + 1)],
                        start=False, stop=(cc == 3))
            for g in (g1, g2):
                pvv = half_t.pop(("v", g))
                vt_st = vst_p.tile([64, 512], F16, tag="vst")
                nc.vector.tensor_copy(vt_st[:], pvv[:])
                vt_c[2 * g] = vt_st[:, 0:256]
                vt_c[2 * g + 1] = vt_st[:, 256:512]

        def t_h2(fn, a, b):
            return lambda: fn(a, b)

        def t_h(fn, x):
            return lambda: fn(x)

        def t_kp(l):
            return lambda: kp(l)

        def t_vpm(l):
            return lambda: vpm(l)

        def t_vpt(l):
            return lambda: vpt(l)

        def t_qp(b, half):
            return lambda: qp(b, half)

        slot_map = {
            0: [t_vpt(0)],
            1: [t_kp(1), t_qp_w(2)],
            2: [t_vpm(1)],
            3: [t_qp_w(3), t_vpt(1)],
            4: [t_kp_w(1)],
            5: [t_vpm_w(1), t_vpt(2)],
            6: [t_vpt(3)],
            7: [t_h(qpw_a, 4)],
            8: [t_h(qpw_b, 4)],
            9: [t_h(kpw_a, 2)],
            10: [t_h(kpw_b, 2)],
            11: [t_h(vpmw_a, 2)],
            12: [t_h(vpmw_b, 2)],
            13: [t_h(qpw_a, 5)],
            14: [t_h(qpw_b, 5), t_vpt(4)],
            15: [t_vpt(5)],
            16: [t_h(kpw_a, 3)],
            17: [t_h(kpw_b, 3)],
            18: [t_h(qpw_a, 6)],
            19: [t_h(qpw_b, 6)],
            20: [t_h(vpmw_a, 3)],
            21: [t_h(vpmw_b, 3)],
            22: [t_vpt(6)],
            23: [t_vpt(7)],
            25: [t_h(qpw_a, 7)],
            26: [t_h(qpw_b, 7)],
        }

        # ---- attention: flat S -> exp -> AV pipeline over all 36 pairs ----
        qT_v = qTq_sb[:].rearrange("p (half c) -> p half c", half=2)
        po_t = {}

        def emit_S(qb, lp):
            q_lo = qT_v[0:64, :, 256 * qb:256 * (qb + 1)]
            q_hi = qT_v[64:128, :, 256 * qb:256 * (qb + 1)]
            ps = ps_s.tile([128, 1024], F32, tag="s")
            c0 = 256 * lp
            nc.tensor.matmul(ps[:, 0:512], kTq_sb[0:64, c0:c0 + 128],
                             q_lo, start=True, stop=True,
                             tile_position=(0, 0))
            nc.tensor.matmul(ps[:, 512:1024], kTq_sb[64:128, c0 + 128:c0 + 256],
                             q_hi, start=True, stop=True,
                             tile_position=(64, 0))
            return ps

        def emit_exp(qb, lp, ps):
            p_sb = p_pool.tile([128, 1024], F16, tag="p")
            nc.scalar.activation(p_sb[:], ps[:], EXP, scale=SCALE)
            if lp == qb:  # diagonal pair: multiplicative causal mask, DVE,
                # after exp so the ACT stream never waits on it
                nc.vector.tensor_mul(p_sb[:], p_sb[:], mask_sb[:])
            return p_sb

        def emit_AV(qb, lp, p_sb, first, last):
            if first:
                po_new = ps_o.tile([65, 512], F32, tag="po")
                po_t[qb] = po_new
            po = po_t[qb]
            for h in range(2):
                l = 2 * lp + h
                nc.tensor.matmul(po[:], v_sb[:, 65 * l:65 * (l + 1)],
                                 p_sb[:, 512 * h:512 * (h + 1)],
                                 start=(first and h == 0), stop=(last and h == 1))
            if last:  # block done
                po = po_t.pop(qb)
                if qb == QB - 1:
                    # final block: three third casts + three idle rings so
                    # all descriptor generations run in parallel (gpsimd's
                    # queue is clean now that early outputs ride sync)
                    o1 = out_p.tile([65, 172], F16, tag="o1")
                    nc.vector.tensor_copy(o1[:], po[:, 0:172])
                    nc.scalar.dma_start(opart[65 * qb:65 * (qb + 1), 0:172],
                                        o1[:])
                    o2 = out_p.tile([65, 172], F16, tag="o2")
                    nc.vector.tensor_copy(o2[:], po[:, 172:344])
                    nc.sync.dma_start(opart[65 * qb:65 * (qb + 1), 172:344],
                                      o2[:])
                    o3 = out_p.tile([65, 168], F16, tag="o3")
                    nc.vector.tensor_copy(o3[:], po[:, 344:512])
                    nc.gpsimd.dma_start(opart[65 * qb:65 * (qb + 1), 344:512],
                                        o3[:])
                else:
                    o_sb = out_p.tile([65, 512], F16, tag="o")
                    nc.vector.tensor_copy(o_sb[:], po[:])
                    # sync ring is idle after the input waves (~27us) and has
                    # no compute sequencer: outputs there keep gpsimd's queues
                    # clean for the teardown's queue walk
                    nc.sync.dma_start(opart[65 * qb:65 * (qb + 1), :],
                                      o_sb[:])

        s_q = deque()   # (qb, lp, first, last, ps) awaiting exp
        e_q = deque()   # (qb, lp, first, last, p_sb) awaiting AV

        def pump():
            if len(s_q) >= 2:
                qb, lp, fi, la, ps = s_q.popleft()
                e_q.append((qb, lp, fi, la, emit_exp(qb, lp, ps)))
            if len(e_q) >= 2:
                qb, lp, fi, la, p_sb = e_q.popleft()
                emit_AV(qb, lp, p_sb, fi, la)

        # preamble projections: what gates S(0,0), plus qp_w(1) — its oth
        # wave-1 data lands ~13-14.5us (scalar assist) before S(0,0) is ready
        # (~15us), so running it here overlaps the DMA wait instead of
        # serializing after pair 0 (the invariant ~1.8us first exp gap)
        kp(0)
        qp(0, 0)
        qp(0, 1)
        vpm(0)
        qp_w(1)

        pairs = []
        for qb in range(QB - 1):
            for lp in range(qb + 1):
                pairs.append((qb, lp, lp == 0, lp == qb))
        for j, lp in enumerate(range(QB - 1, -1, -1)):
            pairs.append((QB - 1, lp, j == 0, j == QB - 1))
        for p_idx, (qb, lp, fi, la) in enumerate(pairs):
            s_q.append((qb, lp, fi, la, emit_S(qb, lp)))
            pump()
            for fn in slot_map.pop(p_idx, []):
                fn()
        assert not slot_map
        while s_q or e_q:
            if s_q:
                qb, lp, fi, la, ps = s_q.popleft()
                e_q.append((qb, lp, fi, la, emit_exp(qb, lp, ps)))
            if e_q:
                qb, lp, fi, la, p_sb = e_q.popleft()
                emit_AV(qb, lp, p_sb, fi, la)

    nc.compile()
    return nc


def _prep_inputs(x, Wk, Wq, Wv):
    """Per-core input marshalling (layout + fp16 cast, no math)."""
    def swz(w):
        # [C, m] -> [128, 4*m]: chunk cc (rows 128cc..) at free cols m*cc..
        m = w.shape[1]
        return np.ascontiguousarray(
            w.reshape(4, 128, m).transpose(1, 0, 2).reshape(128, 4 * m)
        ).astype(np.float16)

    wkk = swz(np.concatenate([Wk, Wk], axis=1))
    wqq = swz(np.concatenate([Wq, Wq], axis=1))
    wv = swz(Wv)
    mask_cache = {}
    in_maps = []
    for core in range(8):
        b, par = core // 2, core % 2
        xT = np.ascontiguousarray(x[b].T).astype(np.float16)   # [C, T]
        tiles = xT.reshape(C, NKT, 128)
        kv = tiles[:, par::2, :].reshape(C, NLOC * 128)
        oth = tiles[:, 1 - par::2, :].reshape(C, NLOC * 128)
        xt_perm = np.ascontiguousarray(np.concatenate([kv, oth], axis=1))

        if par not in mask_cache:
            J = [par, 2 + par, 1 - par, 3 - par]
            m = np.zeros((128, 1024), np.float16)
            ks = np.arange(128)[:, None]
            qr = np.arange(128)[None, :]
            for mi, off in enumerate((par, 2 + par)):
                for s in range(4):
                    cond = (128 * off + ks) <= (128 * J[s] + qr)
                    m[:, 512 * mi + 128 * s:512 * mi + 128 * (s + 1)] = \
                        np.where(cond, 1.0, 0.0).astype(np.float16)
            mask_cache[par] = m
        in_maps.append({"xt": xt_perm, "wkk": wkk, "wqq": wqq, "wv": wv,
                        "masks": mask_cache[par]})
    return in_maps


def _combine(results):
    """Un-permute query columns, sum partials across the core pairs, divide."""
    out = np.empty((B, T, H), np.float32)
    for b in range(4):
        nats = []
        for par in range(2):
            J = [par, 2 + par, 1 - par, 3 - par]
            r = results[2 * b + par]["opart"].astype(np.float32)
            r = r.reshape(QB, 65, 4, 128)
            nat = np.empty_like(r)
            for s in range(4):
                nat[:, :, J[s], :] = r[:, :, s, :]
            nats.append(nat.transpose(1, 0, 2, 3).reshape(65, T))
        num = nats[0][:64] + nats[1][:64]
        den = nats[0][64] + nats[1][64]
        out[b] = (num / den[None, :]).T
    return out


def kernel(x, Wk, Wq, Wv):
    global LAST_RESULTS
    from concourse.bass_utils import run_bass_kernel_spmd

    if "nc" not in _CACHE:
        _CACHE["nc"] = _build_program()
    nc = _CACHE["nc"]

    in_maps = _prep_inputs(np.asarray(x, np.float32), np.asarray(Wk),
                           np.asarray(Wq), np.asarray(Wv))
    trace = bool(int(os.environ.get("ATTN_TRACE", "0")))
    res = run_bass_kernel_spmd(nc, in_maps, core_ids=list(range(8)),
                               trace=trace)
    LAST_RESULTS = res
    return _combine(res.results)


if __name__ == "__main__":
    rng = np.random.default_rng(0)
    x = rng.standard_normal((B, T, C), dtype=np.float32)
    Wk = (rng.standard_normal((C, H)) * C ** -0.5).astype(np.float32)
    Wq = (rng.standard_normal((C, H)) * C ** -0.5).astype(np.float32)
    Wv = (rng.standard_normal((C, H)) * C ** -0.5).astype(np.float32)
    out = kernel(x, Wk, Wq, Wv)
    k = x @ Wk; q = x @ Wq; v = x @ Wv
    s = np.einsum('bqh,bkh->bqk', q, k) * SCALE
    mask = np.tril(np.ones((T, T), dtype=bool))
    s = np.where(mask, s, -np.inf)
    p = np.exp(s - s.max(-1, keepdims=True))
    p /= p.sum(-1, keepdims=True)
    ref = np.einsum('bqk,bkh->bqh', p, v)
    err = np.abs(out - ref).max() / np.abs(ref).max()
    print("rel err vs numpy:", err)



# revision 29
# speedup vs baseline: 1.0655x; 1.0308x over previous
"""Causal single-head attention on 8 Trainium2 NeuronCores.

Problem: x[B=4,T=4096,C=512] fp32, Wk/Wq/Wv[C,H=64] -> out[B,T,H].

Sharding: 2 cores per batch element. Within a pair, the KEY tiles (128 keys
each, 32 tiles) are interleaved by parity: core parity p owns key tiles
{p, p+2, p+4, ...}. Each core computes, for ALL queries of its batch, the
unnormalized partial softmax numerator (sum_k exp(s) * v) and denominator
(sum_k exp(s)) over its own keys only; the host sums the two partials and
divides. exp() without max-subtraction is safe here (scores ~ N(0,1)).

Every core's program is byte-identical (SPMD): causal structure is identical
for both parities, and all parity/batch differences live in the DMA'd data
(xt columns permuted to [own-parity key tiles | rest], additive diag masks,
host un-permutes output columns).

The steady state is a balanced ACT/PE pipeline: back-to-back [128,1024]
exps stream at ~1000ns each (36 of them, one per key-tile-pair x
query-block); per pair the PE owes S (one concurrent row-packed pair,
~213ns) + AV (2 matmuls, ~430ns), leaving ~360ns of slack that the
projection work must fit inside. Everything is scheduled around keeping
the exp stream dense:
  - xt arrives in 256-col per-cc chunks on the sync (kv) / gpsimd (oth)
    rings, first wave spread across all three rings (per-HWDGE-queue BW is
    only ~25GB/s, ring issue rate ~650ns/DMA), ordered by first consumer
  - projections run as deadline-scheduled filler tasks woven between
    attention pairs (512-col MM groups mid-stream to halve LDWEIGHTS
    overhead; 256-col for the latency-critical preamble)
  - S pair matmuls use tile_position row-packing (concurrent, 64-contract)
  - the causal mask is multiplicative fp16, applied post-exp on the DVE so
    the ACT stream never waits on it; block 7 drains diag-first so the
    final pair has no mask on its critical path
  - output partials are fp16, block-major in DRAM (one DMA per block on
    gpsimd); the LAST block is drained as two half casts whose DMAs ride
    the scalar and sync rings so the two HWDGE descriptor generations run
    in parallel off gpsimd's queue walk

Compute fp16 (PE full rate), PSUM accumulation fp32.
"""

import os
from collections import deque
import numpy as np

B, T, C, H = 4, 4096, 512, 64
NKT = T // 128          # 32 natural key tiles per batch
NLOC = NKT // 2         # 16 local key tiles per core
NP = NLOC // 2          # 8 local tile-pairs == 8 query blocks
QB = T // 512           # 8 query blocks
SCALE = float(H) ** -0.5

_CACHE = {}
LAST_RESULTS = None


def _build_program():
    from contextlib import ExitStack
    import concourse.tile as tile
    from concourse import bacc, mybir
    from concourse.masks import make_identity

    F32 = mybir.dt.float32
    F16 = mybir.dt.float16
    EXP = mybir.ActivationFunctionType.Exp

    nc = bacc.Bacc("TRN2", target_bir_lowering=False, debug=False,
                   num_devices=8)

    xt = nc.dram_tensor("xt", (C, T), F16, kind="ExternalInput").ap()
    wkk = nc.dram_tensor("wkk", (128, 512), F16, kind="ExternalInput").ap()
    wqq = nc.dram_tensor("wqq", (128, 512), F16, kind="ExternalInput").ap()
    wv = nc.dram_tensor("wv", (128, 4 * H), F16, kind="ExternalInput").ap()
    masks = nc.dram_tensor("masks", (128, 1024), F16, kind="ExternalInput").ap()
    opart = nc.dram_tensor("opart", (QB * 65, 512), F16,
                           kind="ExternalOutput").ap()

    with tile.TileContext(nc) as tc, ExitStack() as ctx:
        persist = ctx.enter_context(tc.tile_pool(name="persist", bufs=1))
        vst_p = ctx.enter_context(tc.tile_pool(name="vst", bufs=2))
        p_pool = ctx.enter_context(tc.tile_pool(name="pp", bufs=3))
        out_p = ctx.enter_context(tc.tile_pool(name="outp", bufs=4))
        ps_s = ctx.enter_context(tc.tile_pool(name="pss", bufs=2, space="PSUM"))
        ps_o = ctx.enter_context(tc.tile_pool(name="pso", bufs=2, space="PSUM"))
        ps_j = ctx.enter_context(tc.tile_pool(name="psj", bufs=2, space="PSUM"))

        # ---- persistent SBUF ----
        wkk_sb = persist.tile([128, 512], F16)
        wqq_sb = persist.tile([128, 512], F16)
        wv_sb = persist.tile([128, 4 * H], F16)
        mask_sb = persist.tile([128, 1024], F16)
        kTq_sb = persist.tile([128, NLOC * 128], F16)
        qTq_sb = persist.tile([128, T], F16)
        v_sb = persist.tile([128, NLOC * 65], F16)
        ident = persist.tile([64, 64], F16)
        xt_sb = persist.tile([128, 4 * T], F16)

        zeros16 = persist.tile([128, 16], F16)
        warm_sc = persist.tile([64, 64], F16)

        # local init FIRST: all gpsimd-engine work must precede the gpsimd
        # DMA issues (in-order engine queue), or the PE warmup and identity
        # wait ~13us for the oth xt waves to issue
        nc.gpsimd.memset(zeros16[:], 0.0)
        nc.gpsimd.memset(warm_sc[:], 0.0)
        make_identity(nc, ident[:])
        # ones column of [V|1]: out = in*0 + 1 (also triggers ACT table load)
        v_ones = v_sb[:].rearrange("p (l e) -> p l e", e=65)[:, :, 64:65]
        nc.scalar.activation(v_ones, zeros16[:],
                             mybir.ActivationFunctionType.Copy,
                             bias=1.0, scale=0.0)

        # Per-HWDGE-queue bandwidth is only ~25 GB/s and each ring issues
        # one DMA per ~650ns, so the first wave (kv/oth cols 0:256 + weights)
        # is spread across all three rings for minimum time-to-first-S;
        # later chunks follow in consumption order.
        def kvc(w, cc, eng):
            eng.dma_start(
                xt_sb[:, cc * T + 256 * w:cc * T + 256 * (w + 1)],
                xt[128 * cc:128 * (cc + 1), 256 * w:256 * (w + 1)])

        def othc(w, cc, eng):
            c0 = 2048 + 256 * w
            eng.dma_start(xt_sb[:, cc * T + c0:cc * T + c0 + 256],
                          xt[128 * cc:128 * (cc + 1), c0:c0 + 256])

        nc.sync.dma_start(wkk_sb[:, 0:256], wkk[:, 0:256])
        nc.gpsimd.dma_start(wkk_sb[:, 256:512], wkk[:, 256:512])
        nc.scalar.dma_start(wqq_sb[:, 0:256], wqq[:, 0:256])
        nc.scalar.dma_start(wqq_sb[:, 256:512], wqq[:, 256:512])
        kvc(0, 0, nc.sync)
        kvc(0, 1, nc.sync)
        kvc(0, 2, nc.sync)
        nc.sync.dma_start(wv_sb[:], wv[:])
        kvc(0, 3, nc.gpsimd)
        othc(0, 0, nc.gpsimd)
        othc(0, 1, nc.gpsimd)
        othc(0, 2, nc.gpsimd)
        othc(0, 3, nc.gpsimd)
        # scalar-ring assist: ONLY the two last-arriving oth wave-1 chunks
        # (the gate of the first big exp gap), issued before the masks so
        # their data drains first; all descriptor generation completes well
        # before the first exp enters the ACT queue. Do NOT put more here —
        # 8 mid-stream chunks on this ring measured +8us (v5).
        othc(1, 2, nc.scalar)
        othc(1, 3, nc.scalar)
        nc.scalar.dma_start(mask_sb[:, 0:512], masks[:, 0:512])
        nc.scalar.dma_start(mask_sb[:, 512:1024], masks[:, 512:1024])
        othc(3, 2, nc.scalar)
        othc(3, 3, nc.scalar)
        # Ring assignment follows consumer DEADLINES, not data kind: the oth
        # waves feed qp_w whose deadlines are early (slots 0-8), so they get
        # the sync ring (starts issuing immediately); kv waves feed kp_w/vpm_w
        # with much later deadlines and can absorb gpsimd's slower queue.
        for w in range(1, 8):
            for cc in range(4):
                if w in (1, 3) and cc >= 2:
                    continue
                othc(w, cc, nc.sync)
            for cc in range(4):
                kvc(w, cc, nc.gpsimd)

        # PE warmup during the DMA preamble so the HAM clock-gate is released
        pwarm = ps_o.tile([64, 64], F32, tag="po")
        for _w in range(28):
            nc.tensor.matmul(pwarm[:], warm_sc[:], warm_sc[:],
                             start=True, stop=True, skip_group_check=True)

        # ---- projection micro-tasks (256 output columns each) ----
        def warm_fill(n):
            # keepalive matmuls between DMA-gated preamble MMs: the in-order
            # PE queue idles while the next chunk lands; these bridge the
            # HAM activity window so the ramp doesn't run at 1.2 GHz
            for _ in range(n):
                nc.tensor.matmul(pwarm[:], warm_sc[:], warm_sc[:],
                                 start=True, stop=True,
                                 skip_group_check=True)

        def kp(l, fill=0):
            # K^T for local tiles 2l, 2l+1 -> kTq cols 256l..256l+256,
            # duplicated across both partition halves ([Wk|Wk] stationary)
            pkk = ps_j.tile([128, 256], F32, tag="pj")
            for cc in range(4):
                nc.tensor.matmul(
                    pkk[:], wkk_sb[:, 128 * cc:128 * (cc + 1)],
                    xt_sb[:, cc * T + 256 * l:cc * T + 256 * (l + 1)],
                    start=(cc == 0), stop=(cc == 3))
                warm_fill(fill)
            nc.vector.tensor_copy(kTq_sb[:, 256 * l:256 * (l + 1)], pkk[:])

        vt_c = {}

        def vpm(l):
            pvv = ps_j.tile([64, 256], F32, tag="pj")
            for cc in range(4):
                nc.tensor.matmul(
                    pvv[:], wv_sb[:, H * cc:H * (cc + 1)],
                    xt_sb[:, cc * T + 256 * l:cc * T + 256 * (l + 1)],
                    start=(cc == 0), stop=(cc == 3))
            vt_st = vst_p.tile([64, 256], F16, tag="vst")
            nc.vector.tensor_copy(vt_st[:], pvv[:])
            vt_c[l] = vt_st[:]

        def vpt(l):
            vt_st = vt_c.pop(l)
            pv = ps_j.tile([128, 128], F16, tag="pj")
            for j in range(2):
                nc.tensor.transpose(pv[:, 64 * j:64 * (j + 1)],
                                    vt_st[:, 128 * j:128 * (j + 1)], ident[:])
            vv = v_sb[:].rearrange("p (l e) -> p l e", e=65)
            nc.vector.tensor_copy(
                vv[:, 2 * l:2 * l + 2, 0:64],
                pv[:].rearrange("p (j e) -> p j e", e=64))

        def qp(qb, half, fill=0):
            # Q^T for query block qb (kv half or oth half), duplicated
            # across partition halves ([Wq|Wq] stationary)
            off = 2048 * half + 256 * qb
            pqq = ps_j.tile([128, 256], F32, tag="pj")
            for cc in range(4):
                nc.tensor.matmul(
                    pqq[:], wqq_sb[:, 128 * cc:128 * (cc + 1)],
                    xt_sb[:, cc * T + off:cc * T + off + 256],
                    start=(cc == 0), stop=(cc == 3))
                warm_fill(fill)
            nc.vector.tensor_copy(qTq_sb[:, off:off + 256], pqq[:])

        # wide (512-col) task variants: half the LDWEIGHTS per output column
        xt_v = xt_sb[:].rearrange("p (cc half c) -> p cc half c", cc=4, half=2)
        qT_h = qTq_sb[:].rearrange("p (half c) -> p half c", half=2)

        def qp_w(qb):
            # both halves of query block qb in one MM group: moving is the
            # 2-region AP [kv cols | oth cols], out [128, 512]
            pqq = ps_j.tile([128, 512], F32, tag="pj")
            for cc in range(4):
                nc.tensor.matmul(
                    pqq[:], wqq_sb[:, 128 * cc:128 * (cc + 1)],
                    xt_v[:, cc, :, 256 * qb:256 * (qb + 1)],
                    start=(cc == 0), stop=(cc == 3))
            nc.vector.tensor_copy(qT_h[:, :, 256 * qb:256 * (qb + 1)], pqq[:])

        def kp_w(g):
            # K^T for local tiles 4g..4g+3 (kTq cols 512g..512g+512)
            pkk = ps_j.tile([128, 512], F32, tag="pj")
            for cc in range(4):
                nc.tensor.matmul(
                    pkk[:], wkk_sb[:, 128 * cc:128 * (cc + 1)],
                    xt_sb[:, cc * T + 512 * g:cc * T + 512 * (g + 1)],
                    start=(cc == 0), stop=(cc == 3))
            nc.vector.tensor_copy(kTq_sb[:, 512 * g:512 * (g + 1)], pkk[:])

        def vpm_w(g):
            pvv = ps_j.tile([64, 512], F32, tag="pj")
            for cc in range(4):
                nc.tensor.matmul(
                    pvv[:], wv_sb[:, H * cc:H * (cc + 1)],
                    xt_sb[:, cc * T + 512 * g:cc * T + 512 * (g + 1)],
                    start=(cc == 0), stop=(cc == 3))
            vt_st = vst_p.tile([64, 512], F16, tag="vst")
            nc.vector.tensor_copy(vt_st[:], pvv[:])
            vt_c[2 * g] = vt_st[:, 0:256]
            vt_c[2 * g + 1] = vt_st[:, 256:512]

        def t_qp_w(qb):
            return lambda: qp_w(qb)

        def t_kp_w(g):
            return lambda: kp_w(g)

        def t_vpm_w(g):
            return lambda: vpm_w(g)

        # half-granularity wide tasks: 2 MMs per slot (~fits the per-pair PE
        # slack) with the accumulation group and PSUM tile spanning 2 slots
        half_t = {}

        def qpw_a(qb):
            pqq = ps_j.tile([128, 512], F32, tag="pj")
            half_t[("q", qb)] = pqq
            for cc in range(2):
                nc.tensor.matmul(
                    pqq[:], wqq_sb[:, 128 * cc:128 * (cc + 1)],
                    xt_v[:, cc, :, 256 * qb:256 * (qb + 1)],
                    start=(cc == 0), stop=False)

        def qpw_b(qb):
            pqq = half_t.pop(("q", qb))
            for cc in range(2, 4):
                nc.tensor.matmul(
                    pqq[:], wqq_sb[:, 128 * cc:128 * (cc + 1)],
                    xt_v[:, cc, :, 256 * qb:256 * (qb + 1)],
                    start=False, stop=(cc == 3))
            nc.vector.tensor_copy(qT_h[:, :, 256 * qb:256 * (qb + 1)], pqq[:])

        def kpw_a(g):
            pkk = ps_j.tile([128, 512], F32, tag="pj")
            half_t[("k", g)] = pkk
            for cc in range(2):
                nc.tensor.matmul(
                    pkk[:], wkk_sb[:, 128 * cc:128 * (cc + 1)],
                    xt_sb[:, cc * T + 512 * g:cc * T + 512 * (g + 1)],
                    start=(cc == 0), stop=False)

        def kpw_b(g):
            pkk = half_t.pop(("k", g))
            for cc in range(2, 4):
                nc.tensor.matmul(
                    pkk[:], wkk_sb[:, 128 * cc:128 * (cc + 1)],
                    xt_sb[:, cc * T + 512 * g:cc * T + 512 * (g + 1)],
                    start=False, stop=(cc == 3))
            nc.vector.tensor_copy(kTq_sb[:, 512 * g:512 * (g + 1)], pkk[:])

        def vpmw_a(g):
            pvv = ps_j.tile([64, 512], F32, tag="pj")
            half_t[("v", g)] = pvv
            for cc in range(2):
                nc.tensor.matmul(
                    pvv[:], wv_sb[:, H * cc:H * (cc + 1)],
                    xt_sb[:, cc * T + 512 * g:cc * T + 512 * (g + 1)],
                    start=(cc == 0), stop=False)

        def vpmw_b(g):
            pvv = half_t.pop(("v", g))
            for cc in range(2, 4):
                nc.tensor.matmul(
                    pvv[:], wv_sb[:, H * cc:H * (cc + 1)],
                    xt_sb[:, cc * T + 512 * g:cc * T + 512 * (g + 1)],
                    start=False, stop=(cc == 3))
            vt_st = vst_p.tile([64, 512], F16, tag="vst")
            nc.vector.tensor_copy(vt_st[:], pvv[:])
            vt_c[2 * g] = vt_st[:, 0:256]
            vt_c[2 * g + 1] = vt_st[:, 256:512]

        # cc-outer PAIRED wide tasks: two groups' partials per cc so that
        # consecutive matmuls share the stationary W chunk — walrus emits one
        # LDWEIGHTS for a run of same-stationary matmuls (the warmup loop
        # shows this dedup in the trace), halving proj LDWEIGHTS overhead
        def qpw2a(q1, q2):
            for qb in (q1, q2):
                pqq2 = ps_j.tile([128, 512], F32, tag="pj")
                half_t[("q", qb)] = pqq2
            for cc in range(2):
                for qb in (q1, q2):
                    nc.tensor.matmul(
                        half_t[("q", qb)][:],
                        wqq_sb[:, 128 * cc:128 * (cc + 1)],
                        xt_v[:, cc, :, 256 * qb:256 * (qb + 1)],
                        start=(cc == 0), stop=False)

        def qpw2b(q1, q2):
            for cc in range(2, 4):
                for qb in (q1, q2):
                    nc.tensor.matmul(
                        half_t[("q", qb)][:],
                        wqq_sb[:, 128 * cc:128 * (cc + 1)],
                        xt_v[:, cc, :, 256 * qb:256 * (qb + 1)],
                        start=False, stop=(cc == 3))
            for qb in (q1, q2):
                pqq = half_t.pop(("q", qb))
                nc.vector.tensor_copy(qT_h[:, :, 256 * qb:256 * (qb + 1)],
                                      pqq[:])

        def kpw2a(g1, g2):
            for g in (g1, g2):
                pkk2 = ps_j.tile([128, 512], F32, tag="pj")
                half_t[("k", g)] = pkk2
            for cc in range(2):
                for g in (g1, g2):
                    nc.tensor.matmul(
                        half_t[("k", g)][:],
                        wkk_sb[:, 128 * cc:128 * (cc + 1)],
                        xt_sb[:, cc * T + 512 * g:cc * T + 512 * (g + 1)],
                        start=(cc == 0), stop=False)

        def kpw2b(g1, g2):
            for cc in range(2, 4):
                for g in (g1, g2):
                    nc.tensor.matmul(
                        half_t[("k", g)][:],
                        wkk_sb[:, 128 * cc:128 * (cc + 1)],
                        xt_sb[:, cc * T + 512 * g:cc * T + 512 * (g + 1)],
                        start=False, stop=(cc == 3))
            for g in (g1, g2):
                pkk = half_t.pop(("k", g))
                nc.vector.tensor_copy(kTq_sb[:, 512 * g:512 * (g + 1)],
                                      pkk[:])

        def vpmw2a(g1, g2):
            for g in (g1, g2):
                pvv2 = ps_j.tile([64, 512], F32, tag="pj")
                half_t[("v", g)] = pvv2
            for cc in range(2):
                for g in (g1, g2):
                    nc.tensor.matmul(
                        half_t[("v", g)][:],
                        wv_sb[:, H * cc:H * (cc + 1)],
                        xt_sb[:, cc * T + 512 * g:cc * T + 512 * (g + 1)],
                        start=(cc == 0), stop=False)

        def vpmw2b(g1, g2):
            for cc in range(2, 4):
                for g in (g1, g2):
                    nc.tensor.matmul(
                        half_t[("v", g)][:],
                        wv_sb[:, H * cc:H * (cc + 1)],
                        xt_sb[:, cc * T + 512 * g:cc * T + 512 * (g + 1)],
                        start=False, stop=(cc == 3))
            for g in (g1, g2):
                pvv = half_t.pop(("v", g))
                vt_st = vst_p.tile([64, 512], F16, tag="vst")
                nc.vector.tensor_copy(vt_st[:], pvv[:])
                vt_c[2 * g] = vt_st[:, 0:256]
                vt_c[2 * g + 1] = vt_st[:, 256:512]

        def t_h2(fn, a, b):
            return lambda: fn(a, b)

        def t_h(fn, x):
            return lambda: fn(x)

        def t_kp(l):
            return lambda: kp(l)

        def t_vpm(l):
            return lambda: vpm(l)

        def t_vpt(l):
            return lambda: vpt(l)

        def t_qp(b, half):
            return lambda: qp(b, half)

        slot_map = {
            0: [t_vpt(0)],
            1: [t_kp(1), t_qp_w(2)],
            2: [t_vpm(1)],
            3: [t_qp_w(3), t_vpt(1)],
            4: [t_kp_w(1)],
            5: [t_vpm_w(1), t_vpt(2)],
            6: [t_vpt(3)],
            7: [t_h(qpw_a, 4)],
            8: [t_h(qpw_b, 4)],
            9: [t_h(kpw_a, 2)],
            10: [t_h(kpw_b, 2)],
            11: [t_h(vpmw_a, 2)],
            12: [t_h(vpmw_b, 2)],
            13: [t_h(qpw_a, 5)],
            14: [t_h(qpw_b, 5), t_vpt(4)],
            15: [t_vpt(5)],
            16: [t_h(kpw_a, 3)],
            17: [t_h(kpw_b, 3)],
            18: [t_h(qpw_a, 6)],
            19: [t_h(qpw_b, 6)],
            20: [t_h(vpmw_a, 3)],
            21: [t_h(vpmw_b, 3)],
            22: [t_vpt(6)],
            23: [t_vpt(7)],
            25: [t_h(qpw_a, 7)],
            26: [t_h(qpw_b, 7)],
        }

        # ---- attention: flat S -> exp -> AV pipeline over all 36 pairs ----
        qT_v = qTq_sb[:].rearrange("p (half c) -> p half c", half=2)
        po_t = {}

        def emit_S(qb, lp):
            q_lo = qT_v[0:64, :, 256 * qb:256 * (qb + 1)]
            q_hi = qT_v[64:128, :, 256 * qb:256 * (qb + 1)]
            ps = ps_s.tile([128, 1024], F32, tag="s")
            c0 = 256 * lp
            nc.tensor.matmul(ps[:, 0:512], kTq_sb[0:64, c0:c0 + 128],
                             q_lo, start=True, stop=True,
                             tile_position=(0, 0))
            nc.tensor.matmul(ps[:, 512:1024], kTq_sb[64:128, c0 + 128:c0 + 256],
                             q_hi, start=True, stop=True,
                             tile_position=(64, 0))
            return ps

        def emit_exp(qb, lp, ps):
            p_sb = p_pool.tile([128, 1024], F16, tag="p")
            nc.scalar.activation(p_sb[:], ps[:], EXP, scale=SCALE)
            if lp == qb:  # diagonal pair: multiplicative causal mask, DVE,
                # after exp so the ACT stream never waits on it
                nc.vector.tensor_mul(p_sb[:], p_sb[:], mask_sb[:])
            return p_sb

        def emit_AV(qb, lp, p_sb, first, last):
            if first:
                po_new = ps_o.tile([65, 512], F32, tag="po")
                po_t[qb] = po_new
            po = po_t[qb]
            for h in range(2):
                l = 2 * lp + h
                nc.tensor.matmul(po[:], v_sb[:, 65 * l:65 * (l + 1)],
                                 p_sb[:, 512 * h:512 * (h + 1)],
                                 start=(first and h == 0), stop=(last and h == 1))
            if last:  # block done
                po = po_t.pop(qb)
                if qb == QB - 1:
                    # final block: three third casts + three idle rings so
                    # all descriptor generations run in parallel (gpsimd's
                    # queue is clean now that early outputs ride sync)
                    o1 = out_p.tile([65, 172], F16, tag="o1")
                    nc.vector.tensor_copy(o1[:], po[:, 0:172])
                    nc.scalar.dma_start(opart[65 * qb:65 * (qb + 1), 0:172],
                                        o1[:])
                    o2 = out_p.tile([65, 172], F16, tag="o2")
                    nc.vector.tensor_copy(o2[:], po[:, 172:344])
                    nc.sync.dma_start(opart[65 * qb:65 * (qb + 1), 172:344],
                                      o2[:])
                    o3 = out_p.tile([65, 168], F16, tag="o3")
                    nc.vector.tensor_copy(o3[:], po[:, 344:512])
                    nc.gpsimd.dma_start(opart[65 * qb:65 * (qb + 1), 344:512],
                                        o3[:])
                else:
                    o_sb = out_p.tile([65, 512], F16, tag="o")
                    nc.vector.tensor_copy(o_sb[:], po[:])
                    # sync ring is idle after the input waves (~27us) and has
                    # no compute sequencer: outputs there keep gpsimd's queues
                    # clean for the teardown's queue walk
                    nc.sync.dma_start(opart[65 * qb:65 * (qb + 1), :],
                                      o_sb[:])

        s_q = deque()   # (qb, lp, first, last, ps) awaiting exp
        e_q = deque()   # (qb, lp, first, last, p_sb) awaiting AV

        def pump():
            if len(s_q) >= 2:
                qb, lp, fi, la, ps = s_q.popleft()
                e_q.append((qb, lp, fi, la, emit_exp(qb, lp, ps)))
            if len(e_q) >= 3:
                # lag-3 AV: gives the diag pairs' DVE mask-mul one extra
                # exp-period so the in-order PE never stalls behind AV(diag)
                qb, lp, fi, la, p_sb = e_q.popleft()
                emit_AV(qb, lp, p_sb, fi, la)

        # preamble projections: what gates S(0,0), plus qp_w(1) — its oth
        # wave-1 data lands ~13-14.5us (scalar assist) before S(0,0) is ready
        # (~15us), so running it here overlaps the DMA wait instead of
        # serializing after pair 0 (the invariant ~1.8us first exp gap)
        kp(0)
        qp(0, 0)
        qp(0, 1)
        vpm(0)
        qp_w(1)

        pairs = []
        for qb in range(QB - 1):
            for lp in range(qb + 1):
                pairs.append((qb, lp, lp == 0, lp == qb))
        for j, lp in enumerate(range(QB - 1, -1, -1)):
            pairs.append((QB - 1, lp, j == 0, j == QB - 1))
        for p_idx, (qb, lp, fi, la) in enumerate(pairs):
            s_q.append((qb, lp, fi, la, emit_S(qb, lp)))
            pump()
            for fn in slot_map.pop(p_idx, []):
                fn()
        assert not slot_map
        while s_q or e_q:
            if s_q:
                qb, lp, fi, la, ps = s_q.popleft()
                e_q.append((qb, lp, fi, la, emit_exp(qb, lp, ps)))
            if e_q:
                qb, lp, fi, la, p_sb = e_q.popleft()
                emit_AV(qb, lp, p_sb, fi, la)

    nc.compile()
    return nc


def _prep_inputs(x, Wk, Wq, Wv):
    """Per-core input marshalling (layout + fp16 cast, no math)."""
    def swz(w):
        # [C, m] -> [128, 4*m]: chunk cc (rows 128cc..) at free cols m*cc..
        m = w.shape[1]
        return np.ascontiguousarray(
            w.reshape(4, 128, m).transpose(1, 0, 2).reshape(128, 4 * m)
        ).astype(np.float16)

    wkk = swz(np.concatenate([Wk, Wk], axis=1))
    wqq = swz(np.concatenate([Wq, Wq], axis=1))
    wv = swz(Wv)
    mask_cache = {}
    in_maps = []
    for core in range(8):
        b, par = core // 2, core % 2
        xT = np.ascontiguousarray(x[b].T).astype(np.float16)   # [C, T]
        tiles = xT.reshape(C, NKT, 128)
        kv = tiles[:, par::2, :].reshape(C, NLOC * 128)
        oth = tiles[:, 1 - par::2, :].reshape(C, NLOC * 128)
        xt_perm = np.ascontiguousarray(np.concatenate([kv, oth], axis=1))

        if par not in mask_cache:
            J = [par, 2 + par, 1 - par, 3 - par]
            m = np.zeros((128, 1024), np.float16)
            ks = np.arange(128)[:, None]
            qr = np.arange(128)[None, :]
            for mi, off in enumerate((par, 2 + par)):
                for s in range(4):
                    cond = (128 * off + ks) <= (128 * J[s] + qr)
                    m[:, 512 * mi + 128 * s:512 * mi + 128 * (s + 1)] = \
                        np.where(cond, 1.0, 0.0).astype(np.float16)
            mask_cache[par] = m
        in_maps.append({"xt": xt_perm, "wkk": wkk, "wqq": wqq, "wv": wv,
                        "masks": mask_cache[par]})
    return in_maps


def _combine(results):
    """Un-permute query columns, sum partials across the core pairs, divide."""
    out = np.empty((B, T, H), np.float32)
    for b in range(4):
        nats = []
        for par in range(2):
            J = [par, 2 + par, 1 - par, 3 - par]
            r = results[2 * b + par]["opart"].astype(np.float32)
            r = r.reshape(QB, 65, 4, 128)
            nat = np.empty_like(r)
            for s in range(4):
                nat[:, :, J[s], :] = r[:, :, s, :]
            nats.append(nat.transpose(1, 0, 2, 3).reshape(65, T))
        num = nats[0][:64] + nats[1][:64]
        den = nats[0][64] + nats[1][64]
        out[b] = (num / den[None, :]).T
    return out


def kernel(x, Wk, Wq, Wv):
    global LAST_RESULTS
    from concourse.bass_utils import run_bass_kernel_spmd

    if "nc" not in _CACHE:
        _CACHE["nc"] = _build_program()
    nc = _CACHE["nc"]

    in_maps = _prep_inputs(np.asarray(x, np.float32), np.asarray(Wk),
                           np.asarray(Wq), np.asarray(Wv))
    trace = bool(int(os.environ.get("ATTN_TRACE", "0")))
    res = run_bass_kernel_spmd(nc, in_maps, core_ids=list(range(8)),
                               trace=trace)
    LAST_RESULTS = res
    return _combine(res.results)


if __name__ == "__main__":
    rng = np.random.default_rng(0)
    x = rng.standard_normal((B, T, C), dtype=np.float32)
    Wk = (rng.standard_normal((C, H)) * C ** -0.5).astype(np.float32)
    Wq = (rng.standard_normal((C, H)) * C ** -0.5).astype(np.float32)
    Wv = (rng.standard_normal((C, H)) * C ** -0.5).astype(np.float32)
    out = kernel(x, Wk, Wq, Wv)
    k = x @ Wk; q = x @ Wq; v = x @ Wv
    s = np.einsum('bqh,bkh->bqk', q, k) * SCALE
    mask = np.tril(np.ones((T, T), dtype=bool))
    s = np.where(mask, s, -np.inf)
    p = np.exp(s - s.max(-1, keepdims=True))
    p /= p.sum(-1, keepdims=True)
    ref = np.einsum('bqk,bkh->bqh', p, v)
    err = np.abs(out - ref).max() / np.abs(ref).max()
    print("rel err vs numpy:", err)

